# revision 31
# baseline (speedup 1.0000x reference)
"""Trainium2 Bass kernel for AttentionDS (B=4, N=2048, C=768, 12 heads).

Sharding: 8 cores = (batch b in 0..3) x (head-group g in 0..1, 6 heads each).
Each core computes, for its (b, g):
  - q,k projections feature-major (qkT = [768 feats, 2048 toks])
  - v projection token-major ([2048, 384])
  - attention for its 6 heads entirely in S^T = [key, query] layout
    (no transposes anywhere: S^T = K_T^T-contraction-Q_T directly)
  - softmax without max-subtraction (logits are O(6); exp fused with the
    1/8 scale on the ACT engine; denominator = ones-column appended to V)
  - row-split output projection partial ([2048, 768], no bias)
Host side: pre-transposes inputs to bf16, gathers per-core outputs,
sums the two projection partials per batch and adds the bias.
"""
import os
import numpy as np
import ml_dtypes
from contextlib import ExitStack

import concourse.bass as bass
import concourse.tile as tile
from concourse import bacc, mybir
from concourse import bass_utils

F32 = mybir.dt.float32
BF16 = mybir.dt.bfloat16
EXP = mybir.ActivationFunctionType.Exp

B, N, C = 4, 2048, 768
HEADS, HD = 12, 64
HPC = HEADS // 2          # heads per core = 6
FPC = HPC * HD            # features per core = 384
SCALE = HD ** -0.5        # 0.125
KC = C // 128             # 6 contraction chunks
NT = N // 128             # 16 token/key tiles
BF = ml_dtypes.bfloat16

_CACHE = {}


def _build_kernel(debug=False):
    key = ("nc", debug)
    if key in _CACHE:
        return _CACHE[key]
    nc = bacc.Bacc("TRN2", target_bir_lowering=False, debug=False,
                   enable_asserts=False, num_devices=8)
    xT = nc.dram_tensor("xT", [C, N], BF16, kind="ExternalInput").ap()
    wqkT = nc.dram_tensor("wqkT", [C, 2 * FPC], BF16, kind="ExternalInput").ap()
    wvT = nc.dram_tensor("wvT", [C, FPC], BF16, kind="ExternalInput").ap()
    wpT = nc.dram_tensor("wpT", [FPC, C], BF16, kind="ExternalInput").ap()
    qkT_out = nc.dram_tensor("qkT_out", [2 * FPC, N], F32, kind="ExternalOutput").ap()
    v_out = nc.dram_tensor("v_out", [N, FPC], F32, kind="ExternalOutput").ap()
    outp = nc.dram_tensor("outp", [N, C], F32, kind="ExternalOutput").ap()
    dbg = None
    if debug:
        dbg = {
            "p": nc.dram_tensor("dbg_p", [128, 1024], BF16, kind="ExternalOutput").ap(),
            "oa": nc.dram_tensor("dbg_oa", [HD + 1, 1024], F32, kind="ExternalOutput").ap(),
            "bc": nc.dram_tensor("dbg_bc", [HD, 1024], F32, kind="ExternalOutput").ap(),
            "attn": nc.dram_tensor("dbg_attn", [128, N], BF16, kind="ExternalOutput").ap(),
        }

    with tile.TileContext(nc) as tc, ExitStack() as ctx:
        _emit(ctx, tc, xT, wqkT, wvT, wpT, qkT_out, v_out, outp, dbg)
    nc.compile()
    _CACHE[key] = nc
    return nc


def _emit(ctx, tc, xT, wqkT, wvT, wpT, qkT_out, v_out, outp, dbg=None):
    nc = tc.nc

    const = ctx.enter_context(tc.tile_pool(name="const", bufs=1))
    qkp = ctx.enter_context(tc.tile_pool(name="qkp", bufs=1))
    vp = ctx.enter_context(tc.tile_pool(name="vp", bufs=1))
    ap_ = ctx.enter_context(tc.tile_pool(name="ap", bufs=1))
    stage = ctx.enter_context(tc.tile_pool(name="stage", bufs=2))
    ptp = ctx.enter_context(tc.tile_pool(name="ptp", bufs=3))
    npool = ctx.enter_context(tc.tile_pool(name="npool", bufs=2))

    # ---- load inputs ----
    xt = [const.tile([128, N], BF16, name=f"xt{i}") for i in range(KC)]
    wqk = [const.tile([128, 2 * FPC], BF16, name=f"wqk{i}") for i in range(KC)]
    wv = [const.tile([128, FPC], BF16, name=f"wv{i}") for i in range(KC)]
    wp = [const.tile([128, C], BF16, name=f"wp{i}") for i in range(3)]
    for i in range(KC):
        nc.sync.dma_start(xt[i][:], xT[i * 128:(i + 1) * 128, :])
    for i in range(KC):
        nc.sync.dma_start(wv[i][:], wvT[i * 128:(i + 1) * 128, :])
    for i in range(KC):
        nc.sync.dma_start(wqk[i][:], wqkT[i * 128:(i + 1) * 128, :])
    for i in range(3):
        nc.sync.dma_start(wp[i][:], wpT[i * 128:(i + 1) * 128, :])

    qkb = [qkp.tile([128, N], BF16, name=f"qkb{i}") for i in range(KC)]
    vaug = [vp.tile([128, HPC, HD + 1], BF16, name=f"vaug{t}") for t in range(NT)]
    # per-pair attn tiles: head A on partitions 0-63, head B on 64-127
    attnp = [ap_.tile([128, N], BF16, name=f"attnp{p}") for p in range(3)]

    # ---- phase 1: v projection, token-major (first: its DVE copies drain
    # during the qk projection, so the PE never idles at attention entry) ----
    # v[n, j] = sum_c xT[c, n] * wvT[c, j]
    with tc.tile_pool(name="ps2", bufs=4, space="PSUM") as ps2:
        for tt in range(NT):
            pv = ps2.tile([128, FPC], F32, name="pv")
            for ct in range(KC):
                nc.tensor.matmul(pv[:], xt[ct][:, tt * 128:(tt + 1) * 128],
                                 wv[ct][:], start=(ct == 0), stop=(ct == KC - 1))
            vstg = stage.tile([128, FPC], F32, name="vstg")
            nc.vector.tensor_copy(vstg[:], pv[:])
            nc.sync.dma_start(v_out[tt * 128:(tt + 1) * 128, :], vstg[:])
            nc.vector.tensor_copy(
                vaug[tt][:, :, 0:HD],
                vstg[:].rearrange("p (h d) -> p h d", h=HPC))
            nc.vector.memset(vaug[tt][:, :, HD:HD + 1], 1.0)

    # ---- phase 2: qk projection, feature-major ----
    # qkT[j, n] = sum_c wqkT[c, j] * xT[c, n]
    def emit_qkproj(jt, accs):
        acc0, acc1 = accs
        for ct in range(KC):
            lhsT = wqk[ct][:, jt * 128:(jt + 1) * 128]
            for q4 in range(4):
                nc.tensor.matmul(
                    accs[q4 // 2][:, (q4 % 2) * 512:(q4 % 2 + 1) * 512],
                    lhsT, xt[ct][:, q4 * 512:(q4 + 1) * 512],
                    start=(ct == 0), stop=(ct == KC - 1))
        qstg = stage.tile([128, N], F32, name="qstg")
        nc.vector.tensor_copy(qstg[:, 0:1024], acc0[:])
        nc.vector.tensor_copy(qstg[:, 1024:2048], acc1[:])
        nc.vector.tensor_copy(qkb[jt][:], qstg[:])
        nc.sync.dma_start(qkT_out[jt * 128:(jt + 1) * 128, :], qstg[:])

    # only the tiles attention pair 0 needs (jt 0 = q heads 0-1, jt 3 =
    # k heads 0-1) are projected up front; the rest run between attention
    # pairs, borrowing the attention pools' own PSUM slots
    with tc.tile_pool(name="ps1", bufs=2, space="PSUM") as ps1:
        for jt in (0, 3):
            emit_qkproj(jt, [ps1.tile([128, 1024], F32, name="acc0"),
                             ps1.tile([128, 1024], F32, name="acc1")])

    # ---- phase 3: attention, S^T layout ----
    # S_T[key, q] = sum_d kT[d, key] * qT[d, q]; P = exp(S*scale)
    # outT[d_aug, q] = sum_key vaug[key, d_aug] * P_T[key, q]
    with (tc.tile_pool(name="ps_s", bufs=1, space="PSUM") as ps_s,
          tc.tile_pool(name="ps_o", bufs=1, space="PSUM") as ps_o):
        # dense dummy-matmul burst: the PE clock gate (HAM) un-throttles to
        # 2.4GHz only after ~3.4us of CONTINUOUS matmul activity, and the
        # attention phase alone never provides that (its PE stream has sem
        # waits sprinkled through it) - cold 1.2GHz is self-sustaining.
        wu = ps_s.tile([128, 1024], F32, name="sA", tag="sA")
        for i in range(16):
            osl = slice((i % 2) * 512, (i % 2 + 1) * 512)
            nc.tensor.matmul(wu[:, osl], xt[0][:, 0:128], xt[0][:, 0:512],
                             start=True, stop=True)
        for pr in range(3):
            if pr > 0:
                # project the q/k tiles the NEXT pairs need, reusing the
                # attention pools' PSUM slots (keeps the PE warm and the
                # bank budget at 8)
                emit_qkproj(pr, [ps_s.tile([128, 1024], F32, name="sA", tag="sA"),
                                 ps_s.tile([128, 1024], F32, name="sB", tag="sB")])
                emit_qkproj(3 + pr,
                            [ps_o.tile([128, 1024], F32, name="outA", tag="outA"),
                             ps_o.tile([128, 1024], F32, name="outB", tag="outB")])
            hA, hB = 2 * pr, 2 * pr + 1
            qtile, ktile = qkb[pr], qkb[3 + pr]
            for qh in range(2):
                outA = ps_o.tile([HD + 1, 1024], F32, name="outA")
                outB = ps_o.tile([HD + 1, 1024], F32, name="outB")

                def pv(prev):
                    pkt, ppA, ppB = prev
                    st, sp = (pkt == 0), (pkt == NT - 1)
                    for hf in range(2):
                        osl = slice(hf * 512, (hf + 1) * 512)
                        nc.tensor.matmul(outA[:, osl], vaug[pkt][:, hA, :],
                                         ppA[:, osl], start=st, stop=sp)
                        nc.tensor.matmul(outB[:, osl], vaug[pkt][:, hB, :],
                                         ppB[:, osl], start=st, stop=sp)

                # software-pipelined: PV for kt-1 is emitted after QK/exp of
                # kt so the in-order PE never stalls behind an exp it feeds
                prev = None
                for kt in range(NT):
                    ksl = slice(kt * 128, (kt + 1) * 128)
                    sA = ps_s.tile([128, 1024], F32, name="sA")
                    sB = ps_s.tile([128, 1024], F32, name="sB")
                    for hf in range(2):
                        qsl = slice(qh * 1024 + hf * 512, qh * 1024 + (hf + 1) * 512)
                        osl = slice(hf * 512, (hf + 1) * 512)
                        nc.tensor.matmul(sA[:, osl], ktile[0:64, ksl],
                                         qtile[0:64, qsl], start=True, stop=True)
                        nc.tensor.matmul(sB[:, osl], ktile[64:128, ksl],
                                         qtile[64:128, qsl], start=True, stop=True)
                    pA = ptp.tile([128, 1024], BF16, name="pA")
                    pB = ptp.tile([128, 1024], BF16, name="pB")
                    nc.scalar.activation(pA[:], sA[:], EXP, scale=SCALE)
                    nc.scalar.activation(pB[:], sB[:], EXP, scale=SCALE)
                    if dbg is not None and pr == 0 and qh == 0 and kt == 0:
                        nc.sync.dma_start(dbg["p"][:], pA[:])
                    if prev is not None:
                        pv(prev)
                    prev = (kt, pA, pB)
                pv(prev)

                # drain the PSUM accumulators in two back-to-back copies so
                # the banks free within ~2.2us (any PE pause >3.4us at a
                # boundary re-throttles the clock gate); everything else in
                # the normalize runs off-critical-path from the SBUF copies
                qsl = slice(qh * 1024, (qh + 1) * 1024)
                oaA = npool.tile([HD + 1, 1024], F32, name="oaA")
                oaB = npool.tile([HD + 1, 1024], F32, name="oaB")
                nc.vector.tensor_copy(oaA[:], outA[:])
                nc.vector.tensor_copy(oaB[:], outB[:])
                for half, oaX in ((0, oaA), (1, oaB)):
                    dn = npool.tile([1, 1024], F32, name="dn")
                    nc.vector.tensor_copy(dn[:], oaX[HD:HD + 1, :])
                    bc = npool.tile([HD, 1024], F32, name="bc")
                    nc.gpsimd.partition_broadcast(bc[:], dn[:])
                    rcb = npool.tile([HD, 1024], F32, name="rcb")
                    nc.vector.reciprocal_approx_fast(rcb[:], bc[:])
                    dst = attnp[pr][half * HD:(half + 1) * HD, qsl]
                    nc.vector.tensor_mul(dst, oaX[0:HD, :], rcb[:])
                    if dbg is not None and pr == 0 and qh == 0 and half == 0:
                        nc.sync.dma_start(dbg["oa"][:], oaX[:])
                        nc.sync.dma_start(dbg["bc"][:], rcb[:])

    if dbg is not None:
        nc.sync.dma_start(dbg["attn"][:], attnp[0][:])

    # ---- phase 4: output projection (row-split partial, no bias) ----
    # outp[n, j] = sum_f attnT[f, n] * wpT[f, j]
    with tc.tile_pool(name="ps4", bufs=4, space="PSUM") as ps4:
        for tt in range(NT):
            tsl = slice(tt * 128, (tt + 1) * 128)
            o1 = ps4.tile([128, C], F32, name="o1")
            for fc in range(3):
                st, sp = (fc == 0), (fc == 2)
                for nsl in (slice(0, 512), slice(512, 768)):
                    nc.tensor.matmul(o1[:, nsl], attnp[fc][:, tsl],
                                     wp[fc][:, nsl], start=st, stop=sp)
            ostg = stage.tile([128, C], F32, name="ostg")
            nc.vector.tensor_copy(ostg[:], o1[:])
            nc.sync.dma_start(outp[tsl, :], ostg[:])


def kernel(x, Wqkv, Wproj, bproj):
    x = np.asarray(x, dtype=np.float32)
    Wqkv = np.asarray(Wqkv, dtype=np.float32)
    Wproj = np.asarray(Wproj, dtype=np.float32)
    bproj = np.asarray(bproj, dtype=np.float32)

    nc = _build_kernel()

    in_maps = []
    for c in range(8):
        b, g = c // 2, c % 2
        fsl = slice(g * FPC, (g + 1) * FPC)
        xTb = np.ascontiguousarray(x[b].T).astype(BF)
        wqk = np.concatenate([Wqkv[fsl], Wqkv[C + g * FPC:C + (g + 1) * FPC]], 0)
        wqkT = np.ascontiguousarray(wqk.T).astype(BF)
        wvT = np.ascontiguousarray(Wqkv[2 * C + g * FPC:2 * C + (g + 1) * FPC].T).astype(BF)
        wpT = np.ascontiguousarray(Wproj[:, fsl].T).astype(BF)
        in_maps.append({"xT": xTb, "wqkT": wqkT, "wvT": wvT, "wpT": wpT})

    trace = bool(os.environ.get("ATT_TRACE"))
    res = bass_utils.run_bass_kernel_spmd(nc, in_maps, core_ids=list(range(8)),
                                          trace=trace)
    if trace:
        _CACHE["exec_time_ns"] = res.exec_time_ns
        _CACHE["res"] = res
        print(f"HW exec time: {res.exec_time_ns} ns")

    q = np.empty((B, N, C), np.float32)
    k = np.empty((B, N, C), np.float32)
    v = np.empty((B, N, C), np.float32)
    out = np.empty((B, N, C), np.float32)
    for c in range(8):
        b, g = c // 2, c % 2
        fsl = slice(g * FPC, (g + 1) * FPC)
        r = res.results[c]
        q[b, :, fsl] = r["qkT_out"][0:FPC].T
        k[b, :, fsl] = r["qkT_out"][FPC:2 * FPC].T
        v[b, :, fsl] = r["v_out"]
    for b in range(B):
        out[b] = res.results[2 * b]["outp"] + res.results[2 * b + 1]["outp"] + bproj
    return (out, (q, k, v))


# revision 37
# speedup vs baseline: 1.2860x; 1.2860x over previous
"""Trainium2 Bass kernel for AttentionDS (B=4, N=2048, C=768, 12 heads).

Sharding: 8 cores = (batch b in 0..3) x (head-group g in 0..1, 6 heads each).
Each core computes, for its (b, g):
  - q,k projections feature-major (qkT = [768 feats, 2048 toks])
  - v projection token-major ([2048, 384])
  - attention for its 6 heads entirely in S^T = [key, query] layout
    (no transposes anywhere: S^T = K_T^T-contraction-Q_T directly)
  - softmax without max-subtraction (logits are O(6); exp fused with the
    1/8 scale on the ACT engine; denominator = ones-column appended to V)
  - row-split output projection partial ([2048, 768], no bias)
Host side: pre-transposes inputs to bf16, gathers per-core outputs,
sums the two projection partials per batch and adds the bias.
"""
import os
import numpy as np
import ml_dtypes
from contextlib import ExitStack

import concourse.bass as bass
import concourse.tile as tile
from concourse import bacc, mybir
from concourse import bass_utils

F32 = mybir.dt.float32
BF16 = mybir.dt.bfloat16
EXP = mybir.ActivationFunctionType.Exp

B, N, C = 4, 2048, 768
HEADS, HD = 12, 64
HPC = HEADS // 2          # heads per core = 6
FPC = HPC * HD            # features per core = 384
SCALE = HD ** -0.5        # 0.125
KC = C // 128             # 6 contraction chunks
NT = N // 128             # 16 token/key tiles
BF = ml_dtypes.bfloat16

_CACHE = {}


def _build_kernel(debug=False):
    key = ("nc", debug)
    if key in _CACHE:
        return _CACHE[key]
    nc = bacc.Bacc("TRN2", target_bir_lowering=False, debug=False,
                   enable_asserts=False, num_devices=8)
    xT = nc.dram_tensor("xT", [C, N], BF16, kind="ExternalInput").ap()
    wqkT = nc.dram_tensor("wqkT", [C, 2 * FPC], BF16, kind="ExternalInput").ap()
    wvT = nc.dram_tensor("wvT", [C, FPC], BF16, kind="ExternalInput").ap()
    wpT = nc.dram_tensor("wpT", [FPC, C], BF16, kind="ExternalInput").ap()
    qkT_out = nc.dram_tensor("qkT_out", [2 * FPC, N], F32, kind="ExternalOutput").ap()
    v_out = nc.dram_tensor("v_out", [N, FPC], F32, kind="ExternalOutput").ap()
    outp = nc.dram_tensor("outp", [N, C], F32, kind="ExternalOutput").ap()
    dbg = None
    if debug:
        dbg = {
            "p": nc.dram_tensor("dbg_p", [128, 1024], BF16, kind="ExternalOutput").ap(),
            "oa": nc.dram_tensor("dbg_oa", [HD + 1, 1024], F32, kind="ExternalOutput").ap(),
            "bc": nc.dram_tensor("dbg_bc", [HD, 1024], F32, kind="ExternalOutput").ap(),
            "attn": nc.dram_tensor("dbg_attn", [128, N], BF16, kind="ExternalOutput").ap(),
        }

    with tile.TileContext(nc) as tc, ExitStack() as ctx:
        _emit(ctx, tc, xT, wqkT, wvT, wpT, qkT_out, v_out, outp, dbg)
    nc.compile()
    _CACHE[key] = nc
    return nc


def _emit(ctx, tc, xT, wqkT, wvT, wpT, qkT_out, v_out, outp, dbg=None):
    nc = tc.nc

    const = ctx.enter_context(tc.tile_pool(name="const", bufs=1))
    qkp = ctx.enter_context(tc.tile_pool(name="qkp", bufs=1))
    vp = ctx.enter_context(tc.tile_pool(name="vp", bufs=1))
    ap_ = ctx.enter_context(tc.tile_pool(name="ap", bufs=1))
    stage = ctx.enter_context(tc.tile_pool(name="stage", bufs=2))
    ptp = ctx.enter_context(tc.tile_pool(name="ptp", bufs=4))
    npool = ctx.enter_context(tc.tile_pool(name="npool", bufs=2))

    # ---- load inputs ----
    xt = [const.tile([128, N], BF16, name=f"xt{i}") for i in range(KC)]
    wqk = [const.tile([128, 2 * FPC], BF16, name=f"wqk{i}") for i in range(KC)]
    wv = [const.tile([128, FPC], BF16, name=f"wv{i}") for i in range(KC)]
    wp = [const.tile([128, C], BF16, name=f"wp{i}") for i in range(3)]
    for i in range(KC):
        nc.sync.dma_start(xt[i][:], xT[i * 128:(i + 1) * 128, :])
    for i in range(KC):
        nc.sync.dma_start(wv[i][:], wvT[i * 128:(i + 1) * 128, :])
    for i in range(KC):
        nc.sync.dma_start(wqk[i][:], wqkT[i * 128:(i + 1) * 128, :])
    for i in range(3):
        nc.sync.dma_start(wp[i][:], wpT[i * 128:(i + 1) * 128, :])

    qkb = [qkp.tile([128, N], BF16, name=f"qkb{i}") for i in range(KC)]
    vaug = [vp.tile([128, HPC, HD + 1], BF16, name=f"vaug{t}") for t in range(NT)]
    # per-pair attn tiles: head A on partitions 0-63, head B on 64-127
    attnp = [ap_.tile([128, N], BF16, name=f"attnp{p}") for p in range(3)]

    # ---- phase 1: v projection, token-major (first: its DVE copies drain
    # during the qk projection, so the PE never idles at attention entry) ----
    # v[n, j] = sum_c xT[c, n] * wvT[c, j]
    with tc.tile_pool(name="ps2", bufs=4, space="PSUM") as ps2:
        for tt in range(NT):
            pv = ps2.tile([128, FPC], F32, name="pv")
            for ct in range(KC):
                nc.tensor.matmul(pv[:], xt[ct][:, tt * 128:(tt + 1) * 128],
                                 wv[ct][:], start=(ct == 0), stop=(ct == KC - 1))
            vstg = stage.tile([128, FPC], F32, name="vstg")
            nc.vector.tensor_copy(vstg[:], pv[:])
            nc.sync.dma_start(v_out[tt * 128:(tt + 1) * 128, :], vstg[:])
            nc.vector.tensor_copy(
                vaug[tt][:, :, 0:HD],
                vstg[:].rearrange("p (h d) -> p h d", h=HPC))
            nc.vector.memset(vaug[tt][:, :, HD:HD + 1], 1.0)

    # ---- phase 2: qk projection, feature-major ----
    # qkT[j, n] = sum_c wqkT[c, j] * xT[c, n]
    def emit_qkproj(jt, accs):
        acc0, acc1 = accs
        for ct in range(KC):
            lhsT = wqk[ct][:, jt * 128:(jt + 1) * 128]
            for q4 in range(4):
                nc.tensor.matmul(
                    accs[q4 // 2][:, (q4 % 2) * 512:(q4 % 2 + 1) * 512],
                    lhsT, xt[ct][:, q4 * 512:(q4 + 1) * 512],
                    start=(ct == 0), stop=(ct == KC - 1))
        qstg = stage.tile([128, N], F32, name="qstg")
        nc.vector.tensor_copy(qstg[:, 0:1024], acc0[:])
        nc.vector.tensor_copy(qstg[:, 1024:2048], acc1[:])
        nc.vector.tensor_copy(qkb[jt][:], qstg[:])
        nc.sync.dma_start(qkT_out[jt * 128:(jt + 1) * 128, :], qstg[:])

    with tc.tile_pool(name="ps1", bufs=2, space="PSUM") as ps1:
        for jt in (0, 3, 1, 4, 2, 5):
            emit_qkproj(jt, [ps1.tile([128, 1024], F32, name="acc0"),
                             ps1.tile([128, 1024], F32, name="acc1")])

    # ---- phase 3: attention, S^T layout ----
    # S_T[key, q] = sum_d kT[d, key] * qT[d, q]; P = exp(S*scale)
    # outT[d_aug, q] = sum_key vaug[key, d_aug] * P_T[key, q]
    with (tc.tile_pool(name="ps_s", bufs=1, space="PSUM") as ps_s,
          tc.tile_pool(name="ps_o", bufs=1, space="PSUM") as ps_o):
        # dense dummy-matmul burst: the PE clock gate (HAM) un-throttles to
        # 2.4GHz only after ~3.4us of CONTINUOUS matmul activity, and the
        # attention phase alone never provides that (its PE stream has sem
        # waits sprinkled through it) - cold 1.2GHz is self-sustaining.
        wu = ps_s.tile([128, 1024], F32, name="sA", tag="sA")
        for i in range(16):
            osl = slice((i % 2) * 512, (i % 2 + 1) * 512)
            nc.tensor.matmul(wu[:, osl], xt[0][:, 0:128], xt[0][:, 0:512],
                             start=True, stop=True)
        for pr in range(3):
            hA, hB = 2 * pr, 2 * pr + 1
            qtile, ktile = qkb[pr], qkb[3 + pr]
            for qh in range(2):
                outA = ps_o.tile([HD + 1, 1024], F32, name="outA")
                outB = ps_o.tile([HD + 1, 1024], F32, name="outB")

                def pv(prev):
                    pkt, ppA, ppB = prev
                    st, sp = (pkt == 0), (pkt == NT - 1)
                    for hf in range(2):
                        osl = slice(hf * 512, (hf + 1) * 512)
                        nc.tensor.matmul(outA[:, osl], vaug[pkt][:, hA, :],
                                         ppA[:, osl], start=st, stop=sp)
                        nc.tensor.matmul(outB[:, osl], vaug[pkt][:, hB, :],
                                         ppB[:, osl], start=st, stop=sp)

                # software-pipelined depth 2: PV for kt-2 is emitted after
                # QK/exp of kt, so the in-order PE never stalls behind an
                # exp it feeds, and a qh-boundary drain (~2.2us) is hidden
                # behind two QK groups before the first PV needs the banks
                pend = []
                for kt in range(NT):
                    ksl = slice(kt * 128, (kt + 1) * 128)
                    sA = ps_s.tile([128, 1024], F32, name="sA")
                    sB = ps_s.tile([128, 1024], F32, name="sB")
                    for hf in range(2):
                        qsl = slice(qh * 1024 + hf * 512, qh * 1024 + (hf + 1) * 512)
                        osl = slice(hf * 512, (hf + 1) * 512)
                        nc.tensor.matmul(sA[:, osl], ktile[0:64, ksl],
                                         qtile[0:64, qsl], start=True, stop=True)
                        nc.tensor.matmul(sB[:, osl], ktile[64:128, ksl],
                                         qtile[64:128, qsl], start=True, stop=True)
                    pA = ptp.tile([128, 1024], BF16, name="pA")
                    pB = ptp.tile([128, 1024], BF16, name="pB")
                    nc.scalar.activation(pA[:], sA[:], EXP, scale=SCALE)
                    nc.scalar.activation(pB[:], sB[:], EXP, scale=SCALE)
                    if dbg is not None and pr == 0 and qh == 0 and kt == 0:
                        nc.sync.dma_start(dbg["p"][:], pA[:])
                    pend.append((kt, pA, pB))
                    if len(pend) > 2:
                        pv(pend.pop(0))
                for p in pend:
                    pv(p)

                # drain the PSUM accumulators in two back-to-back copies so
                # the banks free within ~2.2us (any PE pause >3.4us at a
                # boundary re-throttles the clock gate); everything else in
                # the normalize runs off-critical-path from the SBUF copies
                qsl = slice(qh * 1024, (qh + 1) * 1024)
                oaA = npool.tile([HD + 1, 1024], F32, name="oaA")
                oaB = npool.tile([HD + 1, 1024], F32, name="oaB")
                nc.vector.tensor_copy(oaA[:], outA[:])
                nc.vector.tensor_copy(oaB[:], outB[:])
                for half, oaX in ((0, oaA), (1, oaB)):
                    dn = npool.tile([1, 1024], F32, name="dn")
                    nc.vector.tensor_copy(dn[:], oaX[HD:HD + 1, :])
                    bc = npool.tile([HD, 1024], F32, name="bc")
                    nc.gpsimd.partition_broadcast(bc[:], dn[:])
                    rcb = npool.tile([HD, 1024], F32, name="rcb")
                    nc.vector.reciprocal_approx_fast(rcb[:], bc[:])
                    dst = attnp[pr][half * HD:(half + 1) * HD, qsl]
                    nc.vector.tensor_mul(dst, oaX[0:HD, :], rcb[:])
                    if dbg is not None and pr == 0 and qh == 0 and half == 0:
                        nc.sync.dma_start(dbg["oa"][:], oaX[:])
                        nc.sync.dma_start(dbg["bc"][:], rcb[:])

    if dbg is not None:
        nc.sync.dma_start(dbg["attn"][:], attnp[0][:])

    # ---- phase 4: output projection (row-split partial, no bias) ----
    # outp[n, j] = sum_f attnT[f, n] * wpT[f, j]
    with tc.tile_pool(name="ps4", bufs=4, space="PSUM") as ps4:
        # re-warm the PE clock gate: the serial normalize chain above leaves
        # the PE idle >3.4us, which re-throttles it to 1.2GHz
        wu2 = ps4.tile([128, C], F32, name="o1", tag="o1")
        for i in range(16):
            nc.tensor.matmul(wu2[:, 0:512], xt[0][:, 0:128], xt[0][:, 0:512],
                             start=True, stop=True)
        for tt in range(NT):
            tsl = slice(tt * 128, (tt + 1) * 128)
            o1 = ps4.tile([128, C], F32, name="o1")
            for fc in range(3):
                st, sp = (fc == 0), (fc == 2)
                for nsl in (slice(0, 512), slice(512, 768)):
                    nc.tensor.matmul(o1[:, nsl], attnp[fc][:, tsl],
                                     wp[fc][:, nsl], start=st, stop=sp)
            ostg = stage.tile([128, C], F32, name="ostg")
            nc.vector.tensor_copy(ostg[:], o1[:])
            nc.sync.dma_start(outp[tsl, :], ostg[:])


def kernel(x, Wqkv, Wproj, bproj):
    x = np.asarray(x, dtype=np.float32)
    Wqkv = np.asarray(Wqkv, dtype=np.float32)
    Wproj = np.asarray(Wproj, dtype=np.float32)
    bproj = np.asarray(bproj, dtype=np.float32)

    nc = _build_kernel()

    in_maps = []
    for c in range(8):
        b, g = c // 2, c % 2
        fsl = slice(g * FPC, (g + 1) * FPC)
        xTb = np.ascontiguousarray(x[b].T).astype(BF)
        wqk = np.concatenate([Wqkv[fsl], Wqkv[C + g * FPC:C + (g + 1) * FPC]], 0)
        wqkT = np.ascontiguousarray(wqk.T).astype(BF)
        wvT = np.ascontiguousarray(Wqkv[2 * C + g * FPC:2 * C + (g + 1) * FPC].T).astype(BF)
        wpT = np.ascontiguousarray(Wproj[:, fsl].T).astype(BF)
        in_maps.append({"xT": xTb, "wqkT": wqkT, "wvT": wvT, "wpT": wpT})

    trace = bool(os.environ.get("ATT_TRACE"))
    res = bass_utils.run_bass_kernel_spmd(nc, in_maps, core_ids=list(range(8)),
                                          trace=trace)
    if trace:
        _CACHE["exec_time_ns"] = res.exec_time_ns
        _CACHE["res"] = res
        print(f"HW exec time: {res.exec_time_ns} ns")

    q = np.empty((B, N, C), np.float32)
    k = np.empty((B, N, C), np.float32)
    v = np.empty((B, N, C), np.float32)
    out = np.empty((B, N, C), np.float32)
    for c in range(8):
        b, g = c // 2, c % 2
        fsl = slice(g * FPC, (g + 1) * FPC)
        r = res.results[c]
        q[b, :, fsl] = r["qkT_out"][0:FPC].T
        k[b, :, fsl] = r["qkT_out"][FPC:2 * FPC].T
        v[b, :, fsl] = r["v_out"]
    for b in range(B):
        out[b] = res.results[2 * b]["outp"] + res.results[2 * b + 1]["outp"] + bproj
    return (out, (q, k, v))


# revision 38
# speedup vs baseline: 1.2913x; 1.0041x over previous
"""Trainium2 Bass kernel for AttentionDS (B=4, N=2048, C=768, 12 heads).

Sharding: 8 cores = (batch b in 0..3) x (head-group g in 0..1, 6 heads each).
Each core computes, for its (b, g):
  - q,k projections feature-major (qkT = [768 feats, 2048 toks])
  - v projection token-major ([2048, 384])
  - attention for its 6 heads entirely in S^T = [key, query] layout
    (no transposes anywhere: S^T = K_T^T-contraction-Q_T directly)
  - softmax without max-subtraction (logits are O(6); exp fused with the
    1/8 scale on the ACT engine; denominator = ones-column appended to V)
  - row-split output projection partial ([2048, 768], no bias)
Host side: pre-transposes inputs to bf16, gathers per-core outputs,
sums the two projection partials per batch and adds the bias.
"""
import os
import numpy as np
import ml_dtypes
from contextlib import ExitStack

import concourse.bass as bass
import concourse.tile as tile
from concourse import bacc, mybir
from concourse import bass_utils

F32 = mybir.dt.float32
BF16 = mybir.dt.bfloat16
EXP = mybir.ActivationFunctionType.Exp

B, N, C = 4, 2048, 768
HEADS, HD = 12, 64
HPC = HEADS // 2          # heads per core = 6
FPC = HPC * HD            # features per core = 384
SCALE = HD ** -0.5        # 0.125
KC = C // 128             # 6 contraction chunks
NT = N // 128             # 16 token/key tiles
BF = ml_dtypes.bfloat16

_CACHE = {}


def _build_kernel(debug=False):
    key = ("nc", debug)
    if key in _CACHE:
        return _CACHE[key]
    nc = bacc.Bacc("TRN2", target_bir_lowering=False, debug=False,
                   enable_asserts=False, num_devices=8)
    xT = nc.dram_tensor("xT", [C, N], BF16, kind="ExternalInput").ap()
    wqkT = nc.dram_tensor("wqkT", [C, 2 * FPC], BF16, kind="ExternalInput").ap()
    wvT = nc.dram_tensor("wvT", [C, FPC], BF16, kind="ExternalInput").ap()
    wpT = nc.dram_tensor("wpT", [FPC, C], BF16, kind="ExternalInput").ap()
    qkT_out = nc.dram_tensor("qkT_out", [2 * FPC, N], F32, kind="ExternalOutput").ap()
    v_out = nc.dram_tensor("v_out", [N, FPC], F32, kind="ExternalOutput").ap()
    outp = nc.dram_tensor("outp", [N, C], F32, kind="ExternalOutput").ap()
    dbg = None
    if debug:
        dbg = {
            "p": nc.dram_tensor("dbg_p", [128, 1024], BF16, kind="ExternalOutput").ap(),
            "oa": nc.dram_tensor("dbg_oa", [HD + 1, 1024], F32, kind="ExternalOutput").ap(),
            "bc": nc.dram_tensor("dbg_bc", [HD, 1024], F32, kind="ExternalOutput").ap(),
            "attn": nc.dram_tensor("dbg_attn", [128, N], BF16, kind="ExternalOutput").ap(),
        }

    with tile.TileContext(nc) as tc, ExitStack() as ctx:
        _emit(ctx, tc, xT, wqkT, wvT, wpT, qkT_out, v_out, outp, dbg)
    nc.compile()
    _CACHE[key] = nc
    return nc


def _emit(ctx, tc, xT, wqkT, wvT, wpT, qkT_out, v_out, outp, dbg=None):
    nc = tc.nc

    const = ctx.enter_context(tc.tile_pool(name="const", bufs=1))
    qkp = ctx.enter_context(tc.tile_pool(name="qkp", bufs=1))
    vp = ctx.enter_context(tc.tile_pool(name="vp", bufs=1))
    ap_ = ctx.enter_context(tc.tile_pool(name="ap", bufs=1))
    stage = ctx.enter_context(tc.tile_pool(name="stage", bufs=2))
    ptp = ctx.enter_context(tc.tile_pool(name="ptp", bufs=4))
    npool = ctx.enter_context(tc.tile_pool(name="npool", bufs=2))

    # ---- load inputs ----
    xt = [const.tile([128, N], BF16, name=f"xt{i}") for i in range(KC)]
    wqk = [const.tile([128, 2 * FPC], BF16, name=f"wqk{i}") for i in range(KC)]
    wv = [const.tile([128, FPC], BF16, name=f"wv{i}") for i in range(KC)]
    wp = [const.tile([128, C], BF16, name=f"wp{i}") for i in range(3)]
    # interleave x and wv loads so the first v-proj matmul can issue as soon
    # as chunk 0 lands instead of waiting for the whole x transfer
    for i in range(KC):
        nc.sync.dma_start(wv[i][:], wvT[i * 128:(i + 1) * 128, :])
        nc.sync.dma_start(xt[i][:], xT[i * 128:(i + 1) * 128, :])
    for i in range(KC):
        nc.sync.dma_start(wqk[i][:], wqkT[i * 128:(i + 1) * 128, :])
    for i in range(3):
        nc.sync.dma_start(wp[i][:], wpT[i * 128:(i + 1) * 128, :])

    qkb = [qkp.tile([128, N], BF16, name=f"qkb{i}") for i in range(KC)]
    vaug = [vp.tile([128, HPC, HD + 1], BF16, name=f"vaug{t}") for t in range(NT)]
    # per-pair attn tiles: head A on partitions 0-63, head B on 64-127
    attnp = [ap_.tile([128, N], BF16, name=f"attnp{p}") for p in range(3)]

    # ---- phase 1: v projection, token-major (first: its DVE copies drain
    # during the qk projection, so the PE never idles at attention entry) ----
    # v[n, j] = sum_c xT[c, n] * wvT[c, j]
    with tc.tile_pool(name="ps2", bufs=4, space="PSUM") as ps2:
        for tt in range(NT):
            pv = ps2.tile([128, FPC], F32, name="pv")
            for ct in range(KC):
                nc.tensor.matmul(pv[:], xt[ct][:, tt * 128:(tt + 1) * 128],
                                 wv[ct][:], start=(ct == 0), stop=(ct == KC - 1))
            vstg = stage.tile([128, FPC], F32, name="vstg")
            nc.vector.tensor_copy(vstg[:], pv[:])
            nc.sync.dma_start(v_out[tt * 128:(tt + 1) * 128, :], vstg[:])
            nc.vector.tensor_copy(
                vaug[tt][:, :, 0:HD],
                vstg[:].rearrange("p (h d) -> p h d", h=HPC))
            nc.vector.memset(vaug[tt][:, :, HD:HD + 1], 1.0)

    # ---- phase 2: qk projection, feature-major ----
    # qkT[j, n] = sum_c wqkT[c, j] * xT[c, n]
    def emit_qkproj(jt, accs):
        acc0, acc1 = accs
        for ct in range(KC):
            lhsT = wqk[ct][:, jt * 128:(jt + 1) * 128]
            for q4 in range(4):
                nc.tensor.matmul(
                    accs[q4 // 2][:, (q4 % 2) * 512:(q4 % 2 + 1) * 512],
                    lhsT, xt[ct][:, q4 * 512:(q4 + 1) * 512],
                    start=(ct == 0), stop=(ct == KC - 1))
        qstg = stage.tile([128, N], F32, name="qstg")
        nc.vector.tensor_copy(qstg[:, 0:1024], acc0[:])
        nc.vector.tensor_copy(qstg[:, 1024:2048], acc1[:])
        nc.vector.tensor_copy(qkb[jt][:], qstg[:])
        nc.sync.dma_start(qkT_out[jt * 128:(jt + 1) * 128, :], qstg[:])

    with tc.tile_pool(name="ps1", bufs=2, space="PSUM") as ps1:
        for jt in (0, 3, 1, 4, 2, 5):
            emit_qkproj(jt, [ps1.tile([128, 1024], F32, name="acc0"),
                             ps1.tile([128, 1024], F32, name="acc1")])

    # ---- phase 3: attention, S^T layout ----
    # S_T[key, q] = sum_d kT[d, key] * qT[d, q]; P = exp(S*scale)
    # outT[d_aug, q] = sum_key vaug[key, d_aug] * P_T[key, q]
    with (tc.tile_pool(name="ps_s", bufs=1, space="PSUM") as ps_s,
          tc.tile_pool(name="ps_o", bufs=1, space="PSUM") as ps_o):
        # dense dummy-matmul burst: the PE clock gate (HAM) un-throttles to
        # 2.4GHz only after ~3.4us of CONTINUOUS matmul activity, and the
        # attention phase alone never provides that (its PE stream has sem
        # waits sprinkled through it) - cold 1.2GHz is self-sustaining.
        wu = ps_s.tile([128, 1024], F32, name="sA", tag="sA")
        for i in range(16):
            osl = slice((i % 2) * 512, (i % 2 + 1) * 512)
            nc.tensor.matmul(wu[:, osl], xt[0][:, 0:128], xt[0][:, 0:512],
                             start=True, stop=True)
        for pr in range(3):
            hA, hB = 2 * pr, 2 * pr + 1
            qtile, ktile = qkb[pr], qkb[3 + pr]
            for qh in range(2):
                outA = ps_o.tile([HD + 1, 1024], F32, name="outA")
                outB = ps_o.tile([HD + 1, 1024], F32, name="outB")

                def pv(prev):
                    pkt, ppA, ppB = prev
                    st, sp = (pkt == 0), (pkt == NT - 1)
                    for hf in range(2):
                        osl = slice(hf * 512, (hf + 1) * 512)
                        nc.tensor.matmul(outA[:, osl], vaug[pkt][:, hA, :],
                                         ppA[:, osl], start=st, stop=sp)
                        nc.tensor.matmul(outB[:, osl], vaug[pkt][:, hB, :],
                                         ppB[:, osl], start=st, stop=sp)

                # software-pipelined depth 2: PV for kt-2 is emitted after
                # QK/exp of kt, so the in-order PE never stalls behind an
                # exp it feeds, and a qh-boundary drain (~2.2us) is hidden
                # behind two QK groups before the first PV needs the banks
                pend = []
                for kt in range(NT):
                    ksl = slice(kt * 128, (kt + 1) * 128)
                    sA = ps_s.tile([128, 1024], F32, name="sA")
                    sB = ps_s.tile([128, 1024], F32, name="sB")
                    for hf in range(2):
                        qsl = slice(qh * 1024 + hf * 512, qh * 1024 + (hf + 1) * 512)
                        osl = slice(hf * 512, (hf + 1) * 512)
                        nc.tensor.matmul(sA[:, osl], ktile[0:64, ksl],
                                         qtile[0:64, qsl], start=True, stop=True)
                        nc.tensor.matmul(sB[:, osl], ktile[64:128, ksl],
                                         qtile[64:128, qsl], start=True, stop=True)
                    pA = ptp.tile([128, 1024], BF16, name="pA")
                    pB = ptp.tile([128, 1024], BF16, name="pB")
                    nc.scalar.activation(pA[:], sA[:], EXP, scale=SCALE)
                    nc.scalar.activation(pB[:], sB[:], EXP, scale=SCALE)
                    if dbg is not None and pr == 0 and qh == 0 and kt == 0:
                        nc.sync.dma_start(dbg["p"][:], pA[:])
                    pend.append((kt, pA, pB))
                    if len(pend) > 2:
                        pv(pend.pop(0))
                for p in pend:
                    pv(p)

                # drain the PSUM accumulators in two back-to-back copies so
                # the banks free within ~2.2us (any PE pause >3.4us at a
                # boundary re-throttles the clock gate); everything else in
                # the normalize runs off-critical-path from the SBUF copies
                qsl = slice(qh * 1024, (qh + 1) * 1024)
                oaA = npool.tile([HD + 1, 1024], F32, name="oaA")
                oaB = npool.tile([HD + 1, 1024], F32, name="oaB")
                nc.vector.tensor_copy(oaA[:], outA[:])
                nc.vector.tensor_copy(oaB[:], outB[:])
                for half, oaX in ((0, oaA), (1, oaB)):
                    dn = npool.tile([1, 1024], F32, name="dn")
                    nc.vector.tensor_copy(dn[:], oaX[HD:HD + 1, :])
                    bc = npool.tile([HD, 1024], F32, name="bc")
                    nc.gpsimd.partition_broadcast(bc[:], dn[:])
                    rcb = npool.tile([HD, 1024], F32, name="rcb")
                    nc.vector.reciprocal_approx_fast(rcb[:], bc[:])
                    dst = attnp[pr][half * HD:(half + 1) * HD, qsl]
                    nc.vector.tensor_mul(dst, oaX[0:HD, :], rcb[:])
                    if dbg is not None and pr == 0 and qh == 0 and half == 0:
                        nc.sync.dma_start(dbg["oa"][:], oaX[:])
                        nc.sync.dma_start(dbg["bc"][:], rcb[:])

    if dbg is not None:
        nc.sync.dma_start(dbg["attn"][:], attnp[0][:])

    # ---- phase 4: output projection (row-split partial, no bias) ----
    # outp[n, j] = sum_f attnT[f, n] * wpT[f, j]
    with tc.tile_pool(name="ps4", bufs=4, space="PSUM") as ps4:
        # re-warm the PE clock gate: the serial normalize chain above leaves
        # the PE idle >3.4us, which re-throttles it to 1.2GHz
        wu2 = ps4.tile([128, C], F32, name="o1", tag="o1")
        for i in range(16):
            nc.tensor.matmul(wu2[:, 0:512], xt[0][:, 0:128], xt[0][:, 0:512],
                             start=True, stop=True)
        for tt in range(NT):
            tsl = slice(tt * 128, (tt + 1) * 128)
            o1 = ps4.tile([128, C], F32, name="o1")
            for fc in range(3):
                st, sp = (fc == 0), (fc == 2)
                for nsl in (slice(0, 512), slice(512, 768)):
                    nc.tensor.matmul(o1[:, nsl], attnp[fc][:, tsl],
                                     wp[fc][:, nsl], start=st, stop=sp)
            ostg = stage.tile([128, C], F32, name="ostg")
            nc.vector.tensor_copy(ostg[:], o1[:])
            nc.sync.dma_start(outp[tsl, :], ostg[:])


def kernel(x, Wqkv, Wproj, bproj):
    x = np.asarray(x, dtype=np.float32)
    Wqkv = np.asarray(Wqkv, dtype=np.float32)
    Wproj = np.asarray(Wproj, dtype=np.float32)
    bproj = np.asarray(bproj, dtype=np.float32)

    nc = _build_kernel()

    in_maps = []
    for c in range(8):
        b, g = c // 2, c % 2
        fsl = slice(g * FPC, (g + 1) * FPC)
        xTb = np.ascontiguousarray(x[b].T).astype(BF)
        wqk = np.concatenate([Wqkv[fsl], Wqkv[C + g * FPC:C + (g + 1) * FPC]], 0)
        wqkT = np.ascontiguousarray(wqk.T).astype(BF)
        wvT = np.ascontiguousarray(Wqkv[2 * C + g * FPC:2 * C + (g + 1) * FPC].T).astype(BF)
        wpT = np.ascontiguousarray(Wproj[:, fsl].T).astype(BF)
        in_maps.append({"xT": xTb, "wqkT": wqkT, "wvT": wvT, "wpT": wpT})

    trace = bool(os.environ.get("ATT_TRACE"))
    res = bass_utils.run_bass_kernel_spmd(nc, in_maps, core_ids=list(range(8)),
                                          trace=trace)
    if trace:
        _CACHE["exec_time_ns"] = res.exec_time_ns
        _CACHE["res"] = res
        print(f"HW exec time: {res.exec_time_ns} ns")

    q = np.empty((B, N, C), np.float32)
    k = np.empty((B, N, C), np.float32)
    v = np.empty((B, N, C), np.float32)
    out = np.empty((B, N, C), np.float32)
    for c in range(8):
        b, g = c // 2, c % 2
        fsl = slice(g * FPC, (g + 1) * FPC)
        r = res.results[c]
        q[b, :, fsl] = r["qkT_out"][0:FPC].T
        k[b, :, fsl] = r["qkT_out"][FPC:2 * FPC].T
        v[b, :, fsl] = r["v_out"]
    for b in range(B):
        out[b] = res.results[2 * b]["outp"] + res.results[2 * b + 1]["outp"] + bproj
    return (out, (q, k, v))
